# revision 10
# baseline (speedup 1.0000x reference)
"""CornerPool module kernel for Trainium2 (Bass/Tile), 8-core batch-parallel.

Model (per sample, C=256, H=W=128):
  t = relu(bn(conv3x3(x, w_t)));  tp = reverse-cummax_H(t)
  l = relu(bn(conv3x3(x, w_l)));  lp = reverse-cummax_W(l)
  b = relu(bn(conv3x3(x, w_b)));  bp = cummax_H(b)
  r = relu(bn(conv3x3(x, w_r)));  rp = cummax_W(r)
  tl = relu(bn3(conv3x3(tp+lp)) + bn1(conv1x1(x)));  out_tl = relu(bn(conv3x3(tl)))
  br = relu(bn3(conv3x3(bp+rp)) + bn1(conv1x1(x)));  out_br = relu(bn(conv3x3(br)))

Strategy: one sample per NeuronCore (B=8). All 3x3 convs use 1D Winograd
F(2,3) along W (2/3 the matmul rows of direct conv): 4 transformed input
streams per column pair (d0-d2, d1+d2, d2-d1, d1-d3) are contracted against
G-transformed weights (folded on host) in 4 PSUM groups of bf16 matmuls
(N=512 = 8 rows x 64 pairs), then recombined (y_even = m0+m1+m2,
y_odd = m1-m2-m3) split across engines: ScalarE copies m1/m2 out of PSUM,
GpSimd forms m1+-m2, VectorE adds m0/m3. All W-paired data lives in
even/odd plane ("paired") layout so every transform/recombine access is
contiguous; final outputs are stored as even/odd blocks and interleaved on
host. W-pools run as a single masked prefix-scan per strip (mask resets the
running max at row starts; values are post-relu so >= 0). H-pools use
shifted-max ladders on GpSimd with inter-strip carries; the reverse-H pool
runs as a short pass interleaved into stage C. Intermediates are bf16.
"""

import numpy as np
import ml_dtypes

_P = 128
_SR = 8            # image rows per strip
_BF = ml_dtypes.bfloat16


def _prep_host(inputs):
    """Fold BN scales into weights, apply the F(2,3) weight transform along
    W, build bf16 lhsT-layout arrays and the f32 bias table."""
    f32 = np.float32

    def scaled(name):
        w = np.asarray(inputs["w_" + name], f32)
        s = np.asarray(inputs["s_" + name], f32)
        return (w * s[:, None, None, None]).astype(np.float64)

    def bias(name):
        return np.asarray(inputs["b_" + name], f32)

    def gtrans(w):
        w0, w1, w2 = w[..., 0], w[..., 1], w[..., 2]
        return np.stack([w0, (w0 + w1 + w2) * 0.5, (w0 - w1 + w2) * 0.5, w2],
                        axis=-1)

    # stage A: [128co, 256ci, 3, 3] -> [128k, ci_t*12 + dy*4 + k, 128co]
    def layA(w):
        g = gtrans(w)
        a = g.transpose(1, 2, 3, 0)
        a = a.reshape(2, 128, 3, 4, 128)
        a = a.transpose(1, 0, 2, 3, 4).reshape(128, 24, 128)
        return np.ascontiguousarray(a.astype(_BF))

    wa = np.stack([layA(scaled(n)) for n in ("t", "l", "b", "r")])

    # stage C: 3x3 -> [128, co_t*12 + dy*4 + k, 128]; 1x1 at 24 + co_t*2 + ci_t
    def layC(w3, w1s):
        g = gtrans(w3)
        a3 = g.transpose(1, 2, 3, 0).reshape(128, 3, 4, 2, 128)
        a3 = a3.transpose(0, 3, 1, 2, 4).reshape(128, 24, 128)
        a1 = w1s[:, :, 0, 0].T.reshape(2, 128, 2, 128).astype(np.float64)
        a1 = a1.transpose(1, 2, 0, 3).reshape(128, 4, 128)
        return np.ascontiguousarray(
            np.concatenate([a3, a1], axis=1).astype(_BF))

    def sc1(name):
        w = np.asarray(inputs["w_" + name], f32)
        s = np.asarray(inputs["s_" + name], f32)
        return w * s[:, None, None, None]

    wc = np.stack([layC(scaled("tl3"), sc1("tl1")),
                   layC(scaled("br3"), sc1("br1"))])

    # stage D: [256co,256ci,3,3] -> [128, co_t, ci_t*12 + dy*4 + k, 128]
    def layD(w):
        g = gtrans(w)
        a = g.transpose(1, 2, 3, 0)
        a = a.reshape(2, 128, 3, 4, 2, 128)
        a = a.transpose(1, 4, 0, 2, 3, 5).reshape(128, 2, 24, 128)
        return np.ascontiguousarray(a.astype(_BF))

    wd = np.stack([layD(scaled("tlo")), layD(scaled("bro"))])

    bias_rows = [bias("t"), bias("l"), bias("b"), bias("r")]
    for n3, n1 in (("tl3", "tl1"), ("br3", "br1")):
        comb = bias(n3) + bias(n1)
        bias_rows += [comb[:128], comb[128:]]
    for n in ("tlo", "bro"):
        bb = bias(n)
        bias_rows += [bb[:128], bb[128:]]
    bias_all = np.ascontiguousarray(np.stack(bias_rows).T).astype(f32)

    return {"wa": wa, "wc": wc, "wd": wd, "bias": bias_all}


def _pad_x_sample(xs, H):
    """[256,H,128] f32 -> paired bf16 [2,128,H+2,2,65]:
    plane 0 = padded even cols 0,2..128; plane 1 = odd cols 1,3..129."""
    xp = np.zeros((2, 128, H + 2, 130), np.float32)
    xp[:, :, 1:H + 1, 1:129] = xs.reshape(2, 128, H, 128)
    xpr = xp.reshape(2, 128, H + 2, 65, 2)
    return np.ascontiguousarray(xpr.transpose(0, 1, 2, 4, 3)).astype(_BF)


def _build(H):
    """Build the Bass module for one core (one sample of height H)."""
    import concourse.bacc as bacc
    import concourse.mybir as mybir
    import concourse.tile as tile

    dt = mybir.dt
    Alu = mybir.AluOpType
    Act = mybir.ActivationFunctionType
    S = H // _SR
    HP = H + 2
    NR = _SR + 2

    nc = bacc.Bacc("TRN2", target_bir_lowering=False, debug=False)

    xpad = nc.dram_tensor("xpad", [2, 128, HP, 2, 65], dt.bfloat16,
                          kind="ExternalInput")
    wa_d = nc.dram_tensor("wa", [4, 128, 24, 128], dt.bfloat16,
                          kind="ExternalInput")
    wc_d = nc.dram_tensor("wc", [2, 128, 28, 128], dt.bfloat16,
                          kind="ExternalInput")
    wd_d = nc.dram_tensor("wd", [2, 128, 2, 24, 128], dt.bfloat16,
                          kind="ExternalInput")
    bias_d = nc.dram_tensor("bias", [128, 12], dt.float32, kind="ExternalInput")
    # outputs in even/odd block layout; host interleaves
    out_tl = nc.dram_tensor("out_tl", [256, H, 2, 64], dt.float32,
                            kind="ExternalOutput")
    out_br = nc.dram_tensor("out_br", [256, H, 2, 64], dt.float32,
                            kind="ExternalOutput")

    # internal DRAM scratch (bf16)
    t_d = nc.dram_tensor("t_s", [128, H, 2, 64], dt.bfloat16)      # block
    lp_d = nc.dram_tensor("lp_s", [128, H, 128], dt.bfloat16)      # interleaved
    sum_d = nc.dram_tensor("sum_s", [2, 128, HP, 2, 65], dt.bfloat16)   # paired
    tlb_d = nc.dram_tensor("tlb_s", [2, 2, 128, HP, 2, 65], dt.bfloat16)

    with tile.TileContext(nc) as tc:
        import contextlib
        with contextlib.ExitStack() as ctx:
            mpool = ctx.enter_context(tc.tile_pool(name="mp", bufs=1))
            cpool = ctx.enter_context(tc.tile_pool(name="cp", bufs=2))
            pspool = ctx.enter_context(tc.tile_pool(name="ps", bufs=8,
                                                    space="PSUM"))

            bt = mpool.tile([128, 12], dt.float32, tag="bias")
            nc.scalar.dma_start(bt[:], bias_d.ap())

            # mask for row-reset prefix scans: 1 everywhere, 0 at row starts
            mk = mpool.tile([128, _SR, 128], dt.bfloat16, tag="mask")
            nc.vector.memset(mk[:], 1.0)
            nc.vector.memset(mk[:, :, 0:1], 0.0)
            mkf = mk[:].rearrange("p a b -> p (a b)")
            # reverse-scan mask: zeros at row ENDS (row starts of the
            # reversed stream)
            mkr = mpool.tile([128, _SR, 128], dt.bfloat16, tag="maskr")
            nc.vector.memset(mkr[:], 1.0)
            nc.vector.memset(mkr[:, :, 127:128], 0.0)
            mkrf = mkr[:].rearrange("p a b -> p (a b)")

            # zero border rows of padded scratch maps
            ztb = mpool.tile([128, 130], dt.bfloat16, tag="zerob")
            nc.vector.memset(ztb[:], 0.0)
            for buf in (sum_d.ap()[0], sum_d.ap()[1],
                        tlb_d.ap()[0, 0], tlb_d.ap()[0, 1],
                        tlb_d.ap()[1, 0], tlb_d.ap()[1, 1]):
                zv = ztb[:].rearrange("p (t g) -> p t g", t=2)
                nc.sync.dma_start(buf[:, 0], zv)
                nc.sync.dma_start(buf[:, HP - 1], zv)

            def load_w(pool, src_ap, nsl, tag, eng=None):
                eng = eng or nc.scalar
                t = pool.tile([128, nsl, 128], dt.bfloat16, tag=tag)
                h = nsl // 2
                eng.dma_start(t[:, :h], src_ap[:, :h])
                eng.dma_start(t[:, h:], src_ap[:, h:])
                return t

            # F(2,3) input transform from a paired strip [128, NR, 2, 65]
            def wtransform(pool, xs, dtag):
                d0 = xs[:, :, 0, 0:64]
                d1 = xs[:, :, 1, 0:64]
                d2 = xs[:, :, 0, 1:65]
                d3 = xs[:, :, 1, 1:65]
                dx = pool.tile([128, 4, NR, 64], dt.bfloat16, tag=dtag)
                nc.gpsimd.tensor_tensor(dx[:, 0], d0, d2, Alu.subtract)
                nc.gpsimd.tensor_tensor(dx[:, 1], d1, d2, Alu.add)
                nc.gpsimd.tensor_tensor(dx[:, 2], d2, d1, Alu.subtract)
                nc.gpsimd.tensor_tensor(dx[:, 3], d1, d3, Alu.subtract)
                return dx

            # 4 m-group matmuls + split recombine -> (y_even, y_odd) bf16
            def wino_mm(ypool, wt, wof, dxs):
                ms = []
                for k in range(4):
                    ps = pspool.tile([128, 512], dt.float32, tag="ps")
                    n = len(dxs) * 3
                    i = 0
                    for ci, dx in enumerate(dxs):
                        for dy in range(3):
                            nc.tensor.matmul(
                                ps[:], wt[:, wof + ci * 12 + dy * 4 + k],
                                dx[:, k, dy:dy + _SR, :],
                                start=(i == 0), stop=(i == n - 1))
                            i += 1
                    ms.append(ps)
                a = ypool.tile([128, 512], dt.bfloat16, tag="ra", bufs=3)
                d = ypool.tile([128, 512], dt.bfloat16, tag="rd", bufs=3)
                nc.scalar.activation(a[:], ms[1][:], Act.Copy)
                nc.scalar.activation(d[:], ms[2][:], Act.Copy)
                b = ypool.tile([128, 512], dt.bfloat16, tag="rb", bufs=3)
                c = ypool.tile([128, 512], dt.bfloat16, tag="rc", bufs=3)
                nc.gpsimd.tensor_tensor(b[:], a[:], d[:], Alu.add)
                nc.gpsimd.tensor_tensor(c[:], a[:], d[:], Alu.subtract)
                ye = ypool.tile([128, _SR, 64], dt.bfloat16, tag="ye", bufs=3)
                yo = ypool.tile([128, _SR, 64], dt.bfloat16, tag="yo", bufs=3)
                nc.vector.tensor_tensor(
                    ye[:].rearrange("p a b -> p (a b)"), b[:], ms[0][:], Alu.add)
                nc.vector.tensor_tensor(
                    yo[:].rearrange("p a b -> p (a b)"), c[:], ms[3][:],
                    Alu.subtract)
                return ye, yo

            def act_block(pool, ye, yo, brow, tag, dtype=dt.bfloat16, bufs=2):
                # -> [128, SR, 2, 64] block tile (plane0 = even output cols)
                t = pool.tile([128, _SR, 2, 64], dtype, tag=tag, bufs=bufs)
                nc.scalar.activation(t[:, :, 0], ye[:], Act.Relu,
                                     bias=bt[:, brow:brow + 1], scale=1.0)
                nc.scalar.activation(t[:, :, 1], yo[:], Act.Relu,
                                     bias=bt[:, brow:brow + 1], scale=1.0)
                return t

            def act_inter(pool, ye, yo, brow, tag, bufs=2):
                # -> [128, SR, 128] interleaved tile (for W-scans)
                t = pool.tile([128, _SR, 128], dt.bfloat16, tag=tag, bufs=bufs)
                tv = t[:].rearrange("p a (g two) -> p a g two", two=2)
                nc.scalar.activation(tv[:, :, :, 0], ye[:], Act.Relu,
                                     bias=bt[:, brow:brow + 1], scale=1.0)
                nc.scalar.activation(tv[:, :, :, 1], yo[:], Act.Relu,
                                     bias=bt[:, brow:brow + 1], scale=1.0)
                return t

            # ---- pass A: 4 convs, W-pools, forward-H pool ------------------
            with contextlib.ExitStack() as actx:
                wpool = actx.enter_context(tc.tile_pool(name="wpA", bufs=1))
                xpool = actx.enter_context(tc.tile_pool(name="xpA", bufs=2))
                dpool = actx.enter_context(tc.tile_pool(name="dpA", bufs=2))
                ypool = actx.enter_context(tc.tile_pool(name="ypA", bufs=3))
                apool = actx.enter_context(tc.tile_pool(name="apA", bufs=2))

                w_a = [load_w(wpool, wa_d.ap()[0], 24, "wa0")]
                carry_b = cpool.tile([128, 1, 2, 64], dt.bfloat16, tag="cryB")
                nc.gpsimd.memset(carry_b[:], 0.0)

                for s in range(S):
                    xs = []
                    for ci in range(2):
                        t = xpool.tile([128, NR, 2, 65], dt.bfloat16,
                                       tag=f"xs{ci}")
                        nc.scalar.dma_start(
                            t[:], xpad.ap()[ci][:, _SR * s:_SR * s + NR])
                        xs.append(t)
                    dxs = [wtransform(dpool, xs[ci], f"dx{ci}")
                           for ci in range(2)]
                    if s == 0:
                        for i in range(1, 4):
                            w_a.append(load_w(wpool, wa_d.ap()[i], 24,
                                              f"wa{i}", eng=nc.sync))

                    # conv T: store raw t map (block layout)
                    ye, yo = wino_mm(ypool, w_a[0], 0, dxs)
                    tb = act_block(apool, ye, yo, 0, "aT")
                    nc.sync.dma_start(t_d.ap()[:, _SR * s:_SR * (s + 1)], tb[:])

                    # conv L: reverse cummax along W (masked scan), store lp
                    ye, yo = wino_mm(ypool, w_a[1], 0, dxs)
                    lb = act_inter(apool, ye, yo, 1, "aL")
                    lf = lb[:].rearrange("p a b -> p (a b)")[:, ::-1]
                    nc.vector.tensor_tensor_scan(lf, mkrf[:, ::-1], lf, 0.0,
                                                 op0=Alu.mult, op1=Alu.max)
                    nc.sync.dma_start(lp_d.ap()[:, _SR * s:_SR * (s + 1), :],
                                      lb[:])

                    # conv B: forward cummax along H (ladder + carry)
                    ye, yo = wino_mm(ypool, w_a[2], 0, dxs)
                    bb = act_block(apool, ye, yo, 2, "aB")
                    nc.vector.tensor_tensor(bb[:, 1:8], bb[:, 1:8], bb[:, 0:7],
                                            Alu.max)
                    nc.vector.tensor_tensor(bb[:, 2:8], bb[:, 2:8], bb[:, 0:6],
                                            Alu.max)
                    nc.vector.tensor_tensor(bb[:, 4:8], bb[:, 4:8], bb[:, 0:4],
                                            Alu.max)
                    nc.vector.tensor_tensor(
                        bb[:], bb[:],
                        carry_b[:].broadcast_to([128, _SR, 2, 64]), Alu.max)
                    if s != S - 1:
                        nxt = cpool.tile([128, 1, 2, 64], dt.bfloat16,
                                         tag="cryB")
                        nc.gpsimd.tensor_copy(nxt[:], bb[:, 7:8])
                        carry_b = nxt

                    # conv R: forward cummax along W, sum with bp -> sum_br
                    ye, yo = wino_mm(ypool, w_a[3], 0, dxs)
                    rb = act_inter(apool, ye, yo, 3, "aR")
                    rf = rb[:].rearrange("p a b -> p (a b)")
                    nc.vector.tensor_tensor_scan(rf, mkf, rf, 0.0,
                                                 op0=Alu.mult, op1=Alu.max)
                    rv = rb[:].rearrange("p a (g two) -> p a g two", two=2)
                    sw = apool.tile([128, _SR, 2, 65], dt.bfloat16, tag="swB")
                    nc.gpsimd.memset(sw[:, :, 0, 0:1], 0.0)
                    nc.gpsimd.memset(sw[:, :, 1, 64:65], 0.0)
                    nc.vector.tensor_tensor(sw[:, :, 1, 0:64], bb[:, :, 0],
                                            rv[:, :, :, 0], Alu.add)
                    nc.vector.tensor_tensor(sw[:, :, 0, 1:65], bb[:, :, 1],
                                            rv[:, :, :, 1], Alu.add)
                    nc.sync.dma_start(
                        sum_d.ap()[1][:, 1 + _SR * s:1 + _SR * (s + 1)], sw[:])

            # ---- stage C (branch br first), with pass A2 interleaved -------
            carry_t = cpool.tile([128, 1, 2, 64], dt.bfloat16, tag="cryT")
            nc.gpsimd.memset(carry_t[:], 0.0)

            def a2_strip(s, pool):
                nonlocal carry_t
                tb = pool.tile([128, _SR, 2, 64], dt.bfloat16, tag="tA2")
                lb = pool.tile([128, _SR, 128], dt.bfloat16, tag="lA2")
                nc.scalar.dma_start(tb[:], t_d.ap()[:, _SR * s:_SR * (s + 1)])
                nc.scalar.dma_start(lb[:],
                                    lp_d.ap()[:, _SR * s:_SR * (s + 1), :])
                nc.vector.tensor_tensor(tb[:, 0:7], tb[:, 0:7], tb[:, 1:8],
                                        Alu.max)
                nc.vector.tensor_tensor(tb[:, 0:6], tb[:, 0:6], tb[:, 2:8],
                                        Alu.max)
                nc.vector.tensor_tensor(tb[:, 0:4], tb[:, 0:4], tb[:, 4:8],
                                        Alu.max)
                nc.vector.tensor_tensor(
                    tb[:], tb[:], carry_t[:].broadcast_to([128, _SR, 2, 64]),
                    Alu.max)
                if s != 0:
                    nxt = cpool.tile([128, 1, 2, 64], dt.bfloat16, tag="cryT")
                    nc.gpsimd.tensor_copy(nxt[:], tb[:, 0:1])
                    carry_t = nxt
                lv = lb[:].rearrange("p a (g two) -> p a g two", two=2)
                sw = pool.tile([128, _SR, 2, 65], dt.bfloat16, tag="swT")
                nc.gpsimd.memset(sw[:, :, 0, 0:1], 0.0)
                nc.gpsimd.memset(sw[:, :, 1, 64:65], 0.0)
                nc.vector.tensor_tensor(sw[:, :, 1, 0:64], tb[:, :, 0],
                                        lv[:, :, :, 0], Alu.add)
                nc.vector.tensor_tensor(sw[:, :, 0, 1:65], tb[:, :, 1],
                                        lv[:, :, :, 1], Alu.add)
                nc.sync.dma_start(
                    sum_d.ap()[0][:, 1 + _SR * s:1 + _SR * (s + 1)], sw[:])

            with contextlib.ExitStack() as cctx:
                wpool = cctx.enter_context(tc.tile_pool(name="wpC", bufs=2))
                xpool = cctx.enter_context(tc.tile_pool(name="xpC", bufs=2))
                dpool = cctx.enter_context(tc.tile_pool(name="dpC", bufs=2))
                ypool = cctx.enter_context(tc.tile_pool(name="ypC", bufs=3))
                apool = cctx.enter_context(tc.tile_pool(name="apC", bufs=2))
                a2pool = cctx.enter_context(tc.tile_pool(name="apA2", bufs=2))
                for bi in (1, 0):
                    w_c = load_w(wpool, wc_d.ap()[bi], 28, "wc")
                    for s in range(S):
                        if bi == 1:
                            a2_strip(S - 1 - s, a2pool)
                        ss = xpool.tile([128, NR, 2, 65], dt.bfloat16, tag="ss")
                        nc.sync.dma_start(
                            ss[:], sum_d.ap()[bi][:, _SR * s:_SR * s + NR])
                        ds = wtransform(dpool, ss, "dsC")
                        xi = []
                        for ci in range(2):
                            t = xpool.tile([128, 2, _SR, 64], dt.bfloat16,
                                           tag=f"xi{ci}")
                            rows = xpad.ap()[ci][:, 1 + _SR * s:1 + _SR * (s + 1)]
                            nc.scalar.dma_start(t[:, 0], rows[:, :, 1, 0:64])
                            nc.scalar.dma_start(t[:, 1], rows[:, :, 0, 1:65])
                            xi.append(t)
                        for co in range(2):
                            c1 = []
                            for pl in range(2):
                                ps = pspool.tile([128, 512], dt.float32,
                                                 tag="ps")
                                for ci in range(2):
                                    nc.tensor.matmul(
                                        ps[:], w_c[:, 24 + co * 2 + ci],
                                        xi[ci][:, pl],
                                        start=(ci == 0), stop=(ci == 1))
                                c1.append(ps)
                            ye, yo = wino_mm(ypool, w_c, co * 12, [ds])
                            nc.vector.tensor_tensor(
                                ye[:].rearrange("p a b -> p (a b)"),
                                ye[:].rearrange("p a b -> p (a b)"),
                                c1[0][:], Alu.add)
                            nc.vector.tensor_tensor(
                                yo[:].rearrange("p a b -> p (a b)"),
                                yo[:].rearrange("p a b -> p (a b)"),
                                c1[1][:], Alu.add)
                            cw = apool.tile([128, _SR, 2, 65], dt.bfloat16,
                                            tag="cw")
                            nc.gpsimd.memset(cw[:, :, 0, 0:1], 0.0)
                            nc.gpsimd.memset(cw[:, :, 1, 64:65], 0.0)
                            brow = 4 + bi * 2 + co
                            nc.scalar.activation(cw[:, :, 1, 0:64], ye[:],
                                                 Act.Relu,
                                                 bias=bt[:, brow:brow + 1],
                                                 scale=1.0)
                            nc.scalar.activation(cw[:, :, 0, 1:65], yo[:],
                                                 Act.Relu,
                                                 bias=bt[:, brow:brow + 1],
                                                 scale=1.0)
                            nc.sync.dma_start(
                                tlb_d.ap()[bi, co][:,
                                                   1 + _SR * s:1 + _SR * (s + 1)],
                                cw[:])

            # ---- stage D: out = relu(wino3x3(tl)), block-layout outputs ----
            with contextlib.ExitStack() as dctx:
                wpool = dctx.enter_context(tc.tile_pool(name="wpD", bufs=2))
                xpool = dctx.enter_context(tc.tile_pool(name="xpD", bufs=2))
                dpool = dctx.enter_context(tc.tile_pool(name="dpD", bufs=2))
                ypool = dctx.enter_context(tc.tile_pool(name="ypD", bufs=3))
                apool = dctx.enter_context(tc.tile_pool(name="apD", bufs=2))
                for bi in (1, 0):
                    w_d = load_w(
                        wpool,
                        wd_d.ap()[bi].rearrange("p a b c -> p (a b) c"),
                        48, "wd")
                    out_d = out_tl if bi == 0 else out_br
                    for s in range(S):
                        dts = []
                        for ci in range(2):
                            t = xpool.tile([128, NR, 2, 65], dt.bfloat16,
                                           tag=f"tl{ci}")
                            nc.sync.dma_start(
                                t[:],
                                tlb_d.ap()[bi, ci][:, _SR * s:_SR * s + NR])
                            dts.append(wtransform(dpool, t, f"dtD{ci}"))
                        for co in range(2):
                            ye, yo = wino_mm(ypool, w_d, co * 24, dts)
                            ot = act_block(apool, ye, yo, 8 + bi * 2 + co,
                                           "oD", dtype=dt.float32)
                            nc.sync.dma_start(
                                out_d.ap()[co * 128:(co + 1) * 128,
                                           _SR * s:_SR * (s + 1)], ot[:])

    nc.compile()
    return nc


_NC_CACHE = {}


def _get_nc(H):
    if H not in _NC_CACHE:
        _NC_CACHE[H] = _build(H)
    return _NC_CACHE[H]


def kernel(**inputs):
    from concourse import bass_utils

    x = np.asarray(inputs["x"], np.float32)
    B, C, H, W = x.shape
    assert (C, W) == (256, 128) and H % _SR == 0

    shared = _prep_host(inputs)
    nc = _get_nc(H)

    in_maps = []
    for b in range(B):
        m = dict(shared)
        m["xpad"] = _pad_x_sample(x[b], H)
        in_maps.append(m)

    import os
    trace = bool(int(os.environ.get("KERNEL_TRACE", "0")))
    res = bass_utils.run_bass_kernel_spmd(
        nc, in_maps, core_ids=list(range(B)), trace=trace)
    kernel.last_result = res

    def deinter(blk):
        # [256, H, 2, 64] block -> [256, H, 128] interleaved
        out = np.empty((256, H, 128), np.float32)
        out[..., 0::2] = blk[..., 0, :]
        out[..., 1::2] = blk[..., 1, :]
        return out

    otl = np.stack([deinter(res.results[b]["out_tl"].reshape(256, H, 2, 64))
                    for b in range(B)])
    obr = np.stack([deinter(res.results[b]["out_br"].reshape(256, H, 2, 64))
                    for b in range(B)])
    return otl, obr


# revision 14
# speedup vs baseline: 1.2898x; 1.2898x over previous
"""CornerPool module kernel for Trainium2 (Bass/Tile), 8-core batch-parallel.

Model (per sample, C=256, H=W=128):
  t = relu(bn(conv3x3(x, w_t)));  tp = reverse-cummax_H(t)
  l = relu(bn(conv3x3(x, w_l)));  lp = reverse-cummax_W(l)
  b = relu(bn(conv3x3(x, w_b)));  bp = cummax_H(b)
  r = relu(bn(conv3x3(x, w_r)));  rp = cummax_W(r)
  tl = relu(bn3(conv3x3(tp+lp)) + bn1(conv1x1(x)));  out_tl = relu(bn(conv3x3(tl)))
  br = relu(bn3(conv3x3(bp+rp)) + bn1(conv1x1(x)));  out_br = relu(bn(conv3x3(br)))

Strategy: one sample per NeuronCore (B=8). All 3x3 convs use 1D Winograd
F(2,3) along W: per output-column pair, 4 transformed input streams
(d0-d2, d1+d2, d2-d1, d1-d3; computed on GpSimd) are contracted against
G-transformed weights (folded on host) in 4 PSUM accumulation groups of
f32r matmuls (N=512 = 8 image rows x 64 column pairs), then recombined
(y0 = m0+m1+m2, y1 = m1-m2-m3) on the Vector engine. This cuts PE matmul
rows to 2/3 of direct conv. The 1x1 convs stay direct. BN scale is folded
into weights, bias applied in the ScalarE relu epilogue.

Pooling is restructured so all four stage-A convs share one forward strip
pass (reusing the transformed x): W-direction pools use the DVE prefix-scan
per row; the bottom (forward-H) pool folds in-pass via a shifted-max ladder
with an inter-strip carry; the top (reverse-H) pool runs as a short reverse
pass over the stored t map. Intermediate maps round-trip DRAM in bf16.
"""

import numpy as np

_P = 128
_SR = 8            # image rows per strip


def _prep_host(inputs):
    """Fold BN scales into weights, apply the F(2,3) weight transform along
    W (G = [[1,0,0],[.5,.5,.5],[.5,-.5,.5],[0,0,1]]), build lhsT-layout
    arrays and the combined bias table."""
    f32 = np.float32

    def scaled(name):
        w = np.asarray(inputs["w_" + name], f32)
        s = np.asarray(inputs["s_" + name], f32)
        return (w * s[:, None, None, None]).astype(np.float64)

    def bias(name):
        return np.asarray(inputs["b_" + name], f32)

    def gtrans(w):
        # w [co, ci, 3dy, 3dx] -> g [co, ci, 3dy, 4k] along dx
        w0, w1, w2 = w[..., 0], w[..., 1], w[..., 2]
        return np.stack([w0, (w0 + w1 + w2) * 0.5, (w0 - w1 + w2) * 0.5, w2],
                        axis=-1)

    # stage A: [128co, 256ci, 3, 3] -> [128k, ci_t*12 + dy*4 + k, 128co]
    def layA(w):
        g = gtrans(w)                                   # [128,256,3,4]
        a = g.transpose(1, 2, 3, 0)                     # [256ci,3dy,4k,128co]
        a = a.reshape(2, 128, 3, 4, 128)                # ci_t,kpart,dy,k,co
        a = a.transpose(1, 0, 2, 3, 4).reshape(128, 24, 128)
        return np.ascontiguousarray(a.astype(f32))

    wa = np.stack([layA(scaled(n)) for n in ("t", "l", "b", "r")])  # [4,128,24,128]

    # stage C: 3x3 [256co,128ci,3,3] -> [128, co_t*12 + dy*4 + k, 128]
    #          1x1 [256co,256ci,1,1] -> [128, 24 + co_t*2 + ci_t, 128]
    def layC(w3, w1s):
        g = gtrans(w3)                                  # [256,128,3,4]
        a3 = g.transpose(1, 2, 3, 0).reshape(128, 3, 4, 2, 128)
        a3 = a3.transpose(0, 3, 1, 2, 4).reshape(128, 24, 128)
        a1 = w1s[:, :, 0, 0].T.reshape(2, 128, 2, 128).astype(np.float64)
        a1 = a1.transpose(1, 2, 0, 3).reshape(128, 4, 128)
        return np.ascontiguousarray(
            np.concatenate([a3, a1], axis=1).astype(f32))

    def sc1(name):
        w = np.asarray(inputs["w_" + name], f32)
        s = np.asarray(inputs["s_" + name], f32)
        return w * s[:, None, None, None]

    wc = np.stack([layC(scaled("tl3"), sc1("tl1")),
                   layC(scaled("br3"), sc1("br1"))])     # [2,128,28,128]

    # stage D: [256co,256ci,3,3] -> [128, co_t, ci_t*12 + dy*4 + k, 128]
    def layD(w):
        g = gtrans(w)                                   # [256,256,3,4]
        a = g.transpose(1, 2, 3, 0)                     # [256ci,3,4,256co]
        a = a.reshape(2, 128, 3, 4, 2, 128)             # ci_t,k,dy,kk,co_t,co
        a = a.transpose(1, 4, 0, 2, 3, 5).reshape(128, 2, 24, 128)
        return np.ascontiguousarray(a.astype(f32))

    wd = np.stack([layD(scaled("tlo")), layD(scaled("bro"))])  # [2,128,2,24,128]

    bias_rows = [bias("t"), bias("l"), bias("b"), bias("r")]
    for n3, n1 in (("tl3", "tl1"), ("br3", "br1")):
        comb = bias(n3) + bias(n1)
        bias_rows += [comb[:128], comb[128:]]
    for n in ("tlo", "bro"):
        bb = bias(n)
        bias_rows += [bb[:128], bb[128:]]
    bias_all = np.ascontiguousarray(np.stack(bias_rows).T).astype(f32)  # [128,12]

    return {"wa": wa, "wc": wc, "wd": wd, "bias": bias_all}


def _pad_x_sample(xs, H):
    """[256,H,128] f32 -> [2,128,H+2,130] zero-padded."""
    xp = np.zeros((2, 128, H + 2, 130), np.float32)
    xp[:, :, 1:H + 1, 1:129] = xs.reshape(2, 128, H, 128)
    return xp


def _build(H):
    """Build the Bass module for one core (one sample of height H)."""
    import concourse.bacc as bacc
    import concourse.mybir as mybir
    import concourse.tile as tile

    dt = mybir.dt
    Alu = mybir.AluOpType
    Act = mybir.ActivationFunctionType
    S = H // _SR          # strips
    HP = H + 2
    NR = _SR + 2          # rows per strip incl. halo

    nc = bacc.Bacc("TRN2", target_bir_lowering=False, debug=False)

    xpad = nc.dram_tensor("xpad", [2, 128, HP, 130], dt.float32, kind="ExternalInput")
    wa_d = nc.dram_tensor("wa", [4, 128, 24, 128], dt.float32, kind="ExternalInput")
    wc_d = nc.dram_tensor("wc", [2, 128, 28, 128], dt.float32, kind="ExternalInput")
    wd_d = nc.dram_tensor("wd", [2, 128, 2, 24, 128], dt.float32, kind="ExternalInput")
    bias_d = nc.dram_tensor("bias", [128, 12], dt.float32, kind="ExternalInput")
    out_tl = nc.dram_tensor("out_tl", [256, H, 128], dt.float32, kind="ExternalOutput")
    out_br = nc.dram_tensor("out_br", [256, H, 128], dt.float32, kind="ExternalOutput")

    # internal DRAM scratch (bf16)
    t_d = nc.dram_tensor("t_s", [128, H, 128], dt.bfloat16)
    lp_d = nc.dram_tensor("lp_s", [128, H, 128], dt.bfloat16)
    sum_d = nc.dram_tensor("sum_s", [2, 128, HP, 130], dt.bfloat16)
    tlb_d = nc.dram_tensor("tlb_s", [2, 2, 128, HP, 130], dt.bfloat16)

    with tile.TileContext(nc) as tc:
        import contextlib
        with contextlib.ExitStack() as ctx:
            mpool = ctx.enter_context(tc.tile_pool(name="mp", bufs=1))
            cpool = ctx.enter_context(tc.tile_pool(name="cp", bufs=2))
            pspool = ctx.enter_context(tc.tile_pool(name="ps", bufs=8, space="PSUM"))

            bt = mpool.tile([128, 12], dt.float32, tag="bias")
            nc.scalar.dma_start(bt[:], bias_d.ap())

            # masks for row-reset prefix scans (values are post-relu >= 0):
            # forward: zeros at row starts; reverse: zeros at row ends
            mk = mpool.tile([128, _SR, 128], dt.bfloat16, tag="mask")
            nc.gpsimd.memset(mk[:], 1.0)
            nc.gpsimd.memset(mk[:, :, 0:1], 0.0)
            mkf = mk[:].rearrange("p a b -> p (a b)")
            mkr = mpool.tile([128, _SR, 128], dt.bfloat16, tag="maskr")
            nc.gpsimd.memset(mkr[:], 1.0)
            nc.gpsimd.memset(mkr[:, :, 127:128], 0.0)
            mkrf = mkr[:].rearrange("p a b -> p (a b)")

            # zero border rows of padded scratch maps
            ztb = mpool.tile([128, 130], dt.bfloat16, tag="zerob")
            nc.vector.memset(ztb[:], 0.0)
            for buf in (sum_d.ap()[0], sum_d.ap()[1],
                        tlb_d.ap()[0, 0], tlb_d.ap()[0, 1],
                        tlb_d.ap()[1, 0], tlb_d.ap()[1, 1]):
                nc.sync.dma_start(buf[:, 0, :], ztb[:])
                nc.sync.dma_start(buf[:, HP - 1, :], ztb[:])

            def load_w(pool, src_ap, nsl, tag, eng=None):
                eng = eng or nc.scalar
                t = pool.tile([128, nsl, 128], dt.float32r, tag=tag)
                h = nsl // 2
                r = src_ap.bitcast(dt.float32r)
                eng.dma_start(t[:, :h], r[:, :h])
                eng.dma_start(t[:, h:], r[:, h:])
                nc.vector.tensor_copy(t[:], t[:].bitcast(dt.float32))
                return t

            # F(2,3) input transform: xs [128, NR, 130] -> dx [128, 4, NR, 64]
            def wtransform(pool, xs, dtag):
                xv = xs.rearrange("p r (g t) -> p r g t", t=2)
                d0 = xv[:, :, 0:64, 0]
                d1 = xv[:, :, 0:64, 1]
                d2 = xv[:, :, 1:65, 0]
                d3 = xv[:, :, 1:65, 1]
                dx = pool.tile([128, 4, NR, 64], dt.float32r, tag=dtag)
                nc.gpsimd.tensor_tensor(dx[:, 0], d0, d2, Alu.subtract)
                nc.gpsimd.tensor_tensor(dx[:, 1], d1, d2, Alu.add)
                nc.gpsimd.tensor_tensor(dx[:, 2], d2, d1, Alu.subtract)
                nc.gpsimd.tensor_tensor(dx[:, 3], d1, d3, Alu.subtract)
                return dx

            # 4 m-group matmuls + recombine -> Y [128, SR, 64, 2] f32r
            def wino_mm(ypool, wt, wof, dxs):
                ms = []
                for k in range(4):
                    ps = pspool.tile([128, 512], dt.float32, tag="ps")
                    n = len(dxs) * 3
                    i = 0
                    for ci, dx in enumerate(dxs):
                        for dy in range(3):
                            nc.tensor.matmul(
                                ps[:], wt[:, wof + ci * 12 + dy * 4 + k],
                                dx[:, k, dy:dy + _SR, :],
                                start=(i == 0), stop=(i == n - 1))
                            i += 1
                    ms.append(ps[:].rearrange("p (r g) -> p r g", g=64))
                y = ypool.tile([128, _SR, 64, 2], dt.float32r, tag="y", bufs=3)
                tc_ = ypool.tile([128, _SR, 64], dt.float32r, tag="yc", bufs=3)
                ta = ypool.tile([128, _SR, 64], dt.float32r, tag="ya", bufs=3)
                tb_ = ypool.tile([128, _SR, 64], dt.float32r, tag="yb", bufs=3)
                nc.scalar.activation(
                    tc_[:].rearrange("p a b -> p (a b)"),
                    ms[1].rearrange("p a b -> p (a b)"), Act.Copy)
                nc.vector.tensor_tensor(ta[:], tc_[:], ms[2], Alu.add)
                nc.vector.tensor_tensor(y[:, :, :, 0], ta[:], ms[0], Alu.add)
                nc.vector.tensor_tensor(tb_[:], tc_[:], ms[2], Alu.subtract)
                nc.vector.tensor_tensor(y[:, :, :, 1], tb_[:], ms[3], Alu.subtract)
                return y

            def act_to(pool, y, brow, tag, dtype=dt.bfloat16, bufs=2):
                t = pool.tile([128, _SR, 128], dtype, tag=tag, bufs=bufs)
                nc.scalar.activation(t[:], y[:],
                                     Act.Relu, bias=bt[:, brow:brow + 1],
                                     scale=1.0)
                return t

            # ---- pass A: 4 convs, W-pools, forward-H pool ------------------
            with contextlib.ExitStack() as actx:
                wpool = actx.enter_context(tc.tile_pool(name="wpA", bufs=1))
                xpool = actx.enter_context(tc.tile_pool(name="xpA", bufs=2))
                dpool = actx.enter_context(tc.tile_pool(name="dpA", bufs=2))
                ypool = actx.enter_context(tc.tile_pool(name="ypA", bufs=3))
                apool = actx.enter_context(tc.tile_pool(name="apA", bufs=2))

                w_a = [load_w(wpool, wa_d.ap()[0], 24, "wa0")]
                carry_b = cpool.tile([128, 1, 128], dt.bfloat16, tag="cryB")
                nc.vector.memset(carry_b[:], 0.0)

                for s in range(S):
                    xs = []
                    for ci in range(2):
                        t = xpool.tile([128, NR, 130], dt.float32, tag=f"xs{ci}")
                        nc.scalar.dma_start(
                            t[:], xpad.ap()[ci][:, _SR * s:_SR * s + NR, :])
                        xs.append(t)
                    dxs = [wtransform(dpool, xs[ci], f"dx{ci}") for ci in range(2)]
                    if s == 0:
                        for i in range(1, 4):
                            w_a.append(load_w(wpool, wa_d.ap()[i], 24, f"wa{i}",
                                              eng=nc.sync))

                    # conv T: store raw t map (bf16)
                    y = wino_mm(ypool, w_a[0], 0, dxs)
                    tb = act_to(apool, y, 0, "aT")
                    nc.sync.dma_start(t_d.ap()[:, _SR * s:_SR * (s + 1), :], tb[:])

                    # conv L: reverse cummax along W, store lp
                    y = wino_mm(ypool, w_a[1], 0, dxs)
                    lb = act_to(apool, y, 1, "aL")
                    lf = lb[:].rearrange("p a b -> p (a b)")[:, ::-1]
                    nc.vector.tensor_tensor_scan(lf, mkrf[:, ::-1], lf, 0.0,
                                                 op0=Alu.mult, op1=Alu.max)
                    nc.sync.dma_start(lp_d.ap()[:, _SR * s:_SR * (s + 1), :], lb[:])

                    # conv B: forward cummax along H (ladder + carry)
                    y = wino_mm(ypool, w_a[2], 0, dxs)
                    bb = act_to(apool, y, 2, "aB")
                    nc.vector.tensor_tensor(bb[:, 1:8], bb[:, 1:8], bb[:, 0:7],
                                            Alu.max)
                    nc.vector.tensor_tensor(bb[:, 2:8], bb[:, 2:8], bb[:, 0:6],
                                            Alu.max)
                    nc.vector.tensor_tensor(bb[:, 4:8], bb[:, 4:8], bb[:, 0:4],
                                            Alu.max)
                    nc.vector.tensor_tensor(bb[:], bb[:],
                                            carry_b[:].broadcast_to(
                                                [128, _SR, 128]), Alu.max)
                    if s != S - 1:
                        nxt = cpool.tile([128, 1, 128], dt.bfloat16, tag="cryB")
                        nc.vector.tensor_copy(nxt[:], bb[:, 7:8])
                        carry_b = nxt

                    # conv R: forward cummax along W, sum with bp -> sum_br
                    y = wino_mm(ypool, w_a[3], 0, dxs)
                    rb = act_to(apool, y, 3, "aR")
                    rf = rb[:].rearrange("p a b -> p (a b)")
                    nc.vector.tensor_tensor_scan(rf, mkf, rf, 0.0,
                                                 op0=Alu.mult, op1=Alu.max)
                    sw = apool.tile([128, _SR, 130], dt.bfloat16, tag="swB")
                    nc.gpsimd.memset(sw[:, :, 0:1], 0.0)
                    nc.gpsimd.memset(sw[:, :, 129:130], 0.0)
                    nc.vector.tensor_tensor(sw[:, :, 1:129], bb[:], rb[:], Alu.add)
                    nc.sync.dma_start(
                        sum_d.ap()[1][:, 1 + _SR * s:1 + _SR * (s + 1), :], sw[:])

            # ---- pass A2 (interleaved into stage C below): reverse-H pool --
            carry_t0 = cpool.tile([128, 1, 128], dt.bfloat16, tag="cryT")
            nc.vector.memset(carry_t0[:], 0.0)
            a2_state = {"carry": carry_t0}

            def a2_strip(s, pool):
                tb = pool.tile([128, _SR, 128], dt.bfloat16, tag="tA2")
                lb = pool.tile([128, _SR, 128], dt.bfloat16, tag="lA2")
                nc.gpsimd.dma_start(tb[:],
                                    t_d.ap()[:, _SR * s:_SR * (s + 1), :])
                nc.gpsimd.dma_start(lb[:],
                                    lp_d.ap()[:, _SR * s:_SR * (s + 1), :])
                nc.vector.tensor_tensor(tb[:, 0:7], tb[:, 0:7], tb[:, 1:8],
                                        Alu.max)
                nc.vector.tensor_tensor(tb[:, 0:6], tb[:, 0:6], tb[:, 2:8],
                                        Alu.max)
                nc.vector.tensor_tensor(tb[:, 0:4], tb[:, 0:4], tb[:, 4:8],
                                        Alu.max)
                nc.vector.tensor_tensor(tb[:], tb[:],
                                        a2_state["carry"][:].broadcast_to(
                                            [128, _SR, 128]), Alu.max)
                if s != 0:
                    nxt = cpool.tile([128, 1, 128], dt.bfloat16, tag="cryT")
                    nc.vector.tensor_copy(nxt[:], tb[:, 0:1])
                    a2_state["carry"] = nxt
                sw = pool.tile([128, _SR, 130], dt.bfloat16, tag="swT")
                nc.gpsimd.memset(sw[:, :, 0:1], 0.0)
                nc.gpsimd.memset(sw[:, :, 129:130], 0.0)
                nc.vector.tensor_tensor(sw[:, :, 1:129], tb[:], lb[:], Alu.add)
                nc.gpsimd.dma_start(
                    sum_d.ap()[0][:, 1 + _SR * s:1 + _SR * (s + 1), :], sw[:])

            # ---- stage C: tl = relu(wino3x3(sum) + conv1x1(x)) -------------
            with contextlib.ExitStack() as cctx:
                wpool = cctx.enter_context(tc.tile_pool(name="wpC", bufs=2))
                xpool = cctx.enter_context(tc.tile_pool(name="xpC", bufs=2))
                dpool = cctx.enter_context(tc.tile_pool(name="dpC", bufs=2))
                ypool = cctx.enter_context(tc.tile_pool(name="ypC", bufs=3))
                apool = cctx.enter_context(tc.tile_pool(name="apC", bufs=2))
                a2pool = cctx.enter_context(tc.tile_pool(name="apA2", bufs=2))
                for bi in (1, 0):
                    w_c = load_w(wpool, wc_d.ap()[bi], 28, "wc")
                    for s in range(S):
                        if bi == 1:
                            a2_strip(S - 1 - s, a2pool)
                        ss = xpool.tile([128, NR, 130], dt.bfloat16, tag="ss")
                        nc.sync.dma_start(
                            ss[:], sum_d.ap()[bi][:, _SR * s:_SR * s + NR, :])
                        ds = wtransform(dpool, ss, "dsC")
                        xi = []
                        for ci in range(2):
                            t = xpool.tile([128, _SR, 128], dt.float32r,
                                           tag=f"xi{ci}")
                            nc.scalar.dma_start(
                                t[:],
                                xpad.ap()[ci][:, 1 + _SR * s:1 + _SR * (s + 1),
                                              1:129].bitcast(dt.float32r))
                            nc.vector.tensor_copy(t[:], t[:].bitcast(dt.float32))
                            xi.append(t)
                        for co in range(2):
                            c1 = []
                            for half in range(2):
                                ps = pspool.tile([128, 512], dt.float32, tag="ps")
                                for ci in range(2):
                                    nc.tensor.matmul(
                                        ps[:], w_c[:, 24 + co * 2 + ci],
                                        xi[ci][:, half * 4:half * 4 + 4, :],
                                        start=(ci == 0), stop=(ci == 1))
                                c1.append(ps[:].rearrange(
                                    "p (r g t) -> p r g t", g=64, t=2))
                            y = wino_mm(ypool, w_c, co * 12, [ds])
                            nc.vector.tensor_tensor(y[:, 0:4], y[:, 0:4], c1[0],
                                                    Alu.add)
                            nc.vector.tensor_tensor(y[:, 4:8], y[:, 4:8], c1[1],
                                                    Alu.add)
                            cw = apool.tile([128, _SR, 130], dt.bfloat16,
                                            tag="cw")
                            nc.gpsimd.memset(cw[:, :, 0:1], 0.0)
                            nc.gpsimd.memset(cw[:, :, 129:130], 0.0)
                            nc.scalar.activation(
                                cw[:, :, 1:129], y[:],
                                Act.Relu,
                                bias=bt[:, 4 + bi * 2 + co:5 + bi * 2 + co],
                                scale=1.0)
                            nc.sync.dma_start(
                                tlb_d.ap()[bi, co][:,
                                                   1 + _SR * s:1 + _SR * (s + 1),
                                                   :], cw[:])

            # ---- stage D: out = relu(wino3x3(tl)) --------------------------
            with contextlib.ExitStack() as dctx:
                wpool = dctx.enter_context(tc.tile_pool(name="wpD", bufs=2))
                xpool = dctx.enter_context(tc.tile_pool(name="xpD", bufs=2))
                dpool = dctx.enter_context(tc.tile_pool(name="dpD", bufs=2))
                ypool = dctx.enter_context(tc.tile_pool(name="ypD", bufs=3))
                apool = dctx.enter_context(tc.tile_pool(name="apD", bufs=2))
                for bi in (1, 0):
                    w_d = load_w(
                        wpool,
                        wd_d.ap()[bi].rearrange("p a b c -> p (a b) c"), 48, "wd")
                    out_d = out_tl if bi == 0 else out_br
                    for s in range(S):
                        dts = []
                        for ci in range(2):
                            t = xpool.tile([128, NR, 130], dt.bfloat16,
                                           tag=f"tl{ci}")
                            nc.sync.dma_start(
                                t[:],
                                tlb_d.ap()[bi, ci][:, _SR * s:_SR * s + NR, :])
                            dts.append(wtransform(dpool, t, f"dtD{ci}"))
                        for co in range(2):
                            y = wino_mm(ypool, w_d, co * 24, dts)
                            ot = act_to(apool, y, 8 + bi * 2 + co, "oD",
                                        dtype=dt.float32)
                            nc.sync.dma_start(
                                out_d.ap()[co * 128:(co + 1) * 128,
                                           _SR * s:_SR * (s + 1), :], ot[:])

    nc.compile()
    return nc


_NC_CACHE = {}


def _get_nc(H):
    if H not in _NC_CACHE:
        _NC_CACHE[H] = _build(H)
    return _NC_CACHE[H]


def kernel(**inputs):
    from concourse import bass_utils

    x = np.asarray(inputs["x"], np.float32)
    B, C, H, W = x.shape
    assert (C, W) == (256, 128) and H % _SR == 0

    shared = _prep_host(inputs)
    nc = _get_nc(H)

    in_maps = []
    for b in range(B):
        m = dict(shared)
        m["xpad"] = _pad_x_sample(x[b], H)
        in_maps.append(m)

    import os
    trace = bool(int(os.environ.get("KERNEL_TRACE", "0")))
    res = bass_utils.run_bass_kernel_spmd(
        nc, in_maps, core_ids=list(range(B)), trace=trace)
    kernel.last_result = res

    otl = np.stack([res.results[b]["out_tl"].reshape(256, H, 128)
                    for b in range(B)])
    obr = np.stack([res.results[b]["out_br"].reshape(256, H, 128)
                    for b in range(B)])
    return otl, obr
